# revision 7
# baseline (speedup 1.0000x reference)
"""Trainium2 Bass kernel for the RetNet-style block (nn_Block_21715354649057).

Strategy (8 NeuronCores, SPMD, two launches, no collectives):
  Launch 1 - shard = (batch b, head-group g of 4 heads). Per core:
    rmsnorm1 (rstd via ones-matmul reduction, weight folded into wq/wk/wv),
    q/k/v projections (fp32r, N=512), chunked retention recurrence (C=128,
    fp32 matmuls, decay masks precomputed on host), partial y @ wo[rows]
    with the skip connection x injected via a flag-identity matmul.
    Outputs transposed partial pT [1024, T] and final state.
  Launch 2 - shard = (batch b, half of T). Per core:
    x2T = p0T + p1T (p0T already includes x), rmsnorm2 (folded into w1),
    MLP: h = x2n @ w1 -> gelu(tanh approx) -> @ w2 (fp32r, N=512, P split in
    quarters), residual restored to token-major via identity matmuls scaled
    by 1/rstd, bias b2 via K=1 matmul. Outputs natural-layout tokens.

All activations flow feature-major ("transposed") so no on-chip transposes
are needed; host pre-transposes x once and weights are consumed in natural
layout.
"""

from contextlib import ExitStack

import numpy as np

import concourse.bass as bass
import concourse.tile as tile
from concourse import bacc, mybir, bass_utils
from concourse.masks import make_identity

F32 = mybir.dt.float32
F32R = mybir.dt.float32r
AF = mybir.ActivationFunctionType

B, T, HD, H, D, P = 4, 4096, 1024, 8, 128, 4096
NH = 4          # heads per launch-1 core
TB = 512        # token block
C = 128         # retention chunk
EPS = 1e-6
KO = HD // 128  # 8 feature tiles

_cache = {}


def _r(ap):
    return ap.bitcast(F32R)


def _split_ko(ap_2d):
    """[1024, N] dram AP -> [128, 8, N] (partition = f % 128, ko = f // 128)."""
    return ap_2d.rearrange("(ko kp) t -> kp ko t", kp=128)


# ---------------------------------------------------------------- launch 1


def build_l1():
    nc = bacc.Bacc("TRN2", target_bir_lowering=False, debug=False)
    xt_d = nc.dram_tensor("xT", [HD, T], F32, kind="ExternalInput")
    wq_d = nc.dram_tensor("wqp", [HD, NH * D], F32, kind="ExternalInput")
    wk_d = nc.dram_tensor("wkp", [HD, NH * D], F32, kind="ExternalInput")
    wv_d = nc.dram_tensor("wvp", [HD, NH * D], F32, kind="ExternalInput")
    wo_d = nc.dram_tensor("wop", [NH * D, HD], F32, kind="ExternalInput")
    s0_d = nc.dram_tensor("s0", [D, NH * D], F32, kind="ExternalInput")
    qsm_d = nc.dram_tensor("qsm", [128, NH, TB], F32, kind="ExternalInput")
    mpr_d = nc.dram_tensor("mpr", [128, NH * C], F32, kind="ExternalInput")
    ktd_d = nc.dram_tensor("ktd", [128, NH * D], F32, kind="ExternalInput")
    gi_d = nc.dram_tensor("gI", [D, NH * D], F32, kind="ExternalInput")
    fi_d = nc.dram_tensor("fI", [128, 128], F32, kind="ExternalInput")
    pt_d = nc.dram_tensor("pT", [HD, T], F32, kind="ExternalOutput")
    so_d = nc.dram_tensor("sout", [D, NH * D], F32, kind="ExternalOutput")

    nblk = T // TB
    with tile.TileContext(nc) as tc, ExitStack() as ctx:
        cst = ctx.enter_context(tc.tile_pool(name="cst", bufs=1))
        xt_p = ctx.enter_context(tc.tile_pool(name="xt", bufs=2))
        xn_p = ctx.enter_context(tc.tile_pool(name="xn", bufs=1))
        sm_p = ctx.enter_context(tc.tile_pool(name="sm", bufs=2))
        qk_p = ctx.enter_context(tc.tile_pool(name="qk", bufs=1))
        yt_p = ctx.enter_context(tc.tile_pool(name="yt", bufs=1))
        pt_p = ctx.enter_context(tc.tile_pool(name="pt", bufs=3))
        ps_pr = ctx.enter_context(tc.tile_pool(name="ps_pr", bufs=2, space="PSUM"))
        ps_rt = ctx.enter_context(tc.tile_pool(name="ps_rt", bufs=3, space="PSUM"))
        ps_wo = ctx.enter_context(tc.tile_pool(name="ps_wo", bufs=2, space="PSUM"))
        ps_rw = ctx.enter_context(tc.tile_pool(name="ps_rw", bufs=1, space="PSUM"))

        # constants
        wq_t = cst.tile([128, KO, NH * D], F32R)
        wk_t = cst.tile([128, KO, NH * D], F32R)
        wv_t = cst.tile([128, KO, NH * D], F32R)
        nc.sync.dma_start(out=wq_t, in_=_r(_split_ko(wq_d.ap())))
        nc.sync.dma_start(out=wk_t, in_=_r(_split_ko(wk_d.ap())))
        nc.sync.dma_start(out=wv_t, in_=_r(_split_ko(wv_d.ap())))
        wo_t = cst.tile([128, NH, HD], F32R)
        nc.sync.dma_start(
            out=wo_t, in_=_r(wo_d.ap().rearrange("(eo kp) f -> kp eo f", kp=128))
        )
        qsm_t = cst.tile([128, NH, TB], F32)
        nc.sync.dma_start(out=qsm_t, in_=qsm_d.ap())
        mpr_t = cst.tile([128, NH * C], F32)
        nc.sync.dma_start(out=mpr_t, in_=mpr_d.ap())
        ktd_t = cst.tile([128, NH * D], F32)
        nc.sync.dma_start(out=ktd_t, in_=ktd_d.ap())
        gi_t = cst.tile([D, NH * D], F32)
        nc.sync.dma_start(out=gi_t, in_=gi_d.ap())
        fi_t = cst.tile([128, 128], F32R)
        nc.sync.dma_start(out=fi_t, in_=_r(fi_d.ap()))
        ones_c = cst.tile([128, 1], F32)
        nc.vector.memset(ones_c, 1.0)
        ones_cr = cst.tile([128, 1], F32R)
        nc.vector.tensor_copy(out=ones_cr, in_=ones_c)
        ones_row = cst.tile([1, 128], F32)
        nc.vector.memset(ones_row, 1.0)
        eps_t = cst.tile([1, 1], F32)
        nc.vector.memset(eps_t, EPS)

        s_a = cst.tile([D, NH * D], F32, tag="s_a")
        s_b = cst.tile([D, NH * D], F32, tag="s_b")
        nc.sync.dma_start(out=s_a, in_=s0_d.ap())
        s_cur, s_nxt = s_a, s_b

        for j in range(nblk):
            tsl = slice(j * TB, (j + 1) * TB)
            xt = xt_p.tile([128, KO, TB], F32R, tag="xt")
            nc.sync.dma_start(out=xt, in_=_r(_split_ko(xt_d.ap()[:, tsl])))

            # --- rstd over features (ones-matmul reduction)
            ssq = ps_rw.tile([1, TB], F32, tag="row")
            for ko in range(KO):
                xsq = sm_p.tile([128, TB], F32R, tag="xsq")
                nc.vector.tensor_mul(out=xsq, in0=xt[:, ko, :], in1=xt[:, ko, :])
                nc.tensor.matmul(
                    ssq, ones_cr, xsq, start=(ko == 0), stop=(ko == KO - 1)
                )
            istd = sm_p.tile([1, TB], F32, tag="istd")
            nc.scalar.activation(
                out=istd, in_=ssq, func=AF.Sqrt, bias=eps_t, scale=1.0 / HD
            )
            rstd = sm_p.tile([1, TB], F32, tag="rstd")
            nc.vector.reciprocal(out=rstd, in_=istd)
            rbc = ps_rw.tile([128, TB], F32, tag="row")
            nc.tensor.matmul(rbc, ones_row, rstd, start=True, stop=True)
            xn = xn_p.tile([128, KO, TB], F32R, tag="xn")
            for ko in range(KO):
                nc.vector.tensor_mul(out=xn[:, ko, :], in0=xt[:, ko, :], in1=rbc)

            # --- projections: Qs & Kd (d-major per head), Ktok & V (token-major)
            qs = qk_p.tile([D, NH, TB], F32, tag="qs")
            kd = qk_p.tile([D, NH, TB], F32, tag="kd")
            for h in range(NH):
                hsl = slice(h * D, (h + 1) * D)
                q_ps = ps_pr.tile([D, TB], F32, tag="proj")
                for ko in range(KO):
                    nc.tensor.matmul(
                        q_ps, wq_t[:, ko, hsl], xn[:, ko, :],
                        start=(ko == 0), stop=(ko == KO - 1),
                    )
                nc.vector.tensor_mul(out=qs[:, h, :], in0=q_ps, in1=qsm_t[:, h, :])
                k_ps = ps_pr.tile([D, TB], F32, tag="proj")
                for ko in range(KO):
                    nc.tensor.matmul(
                        k_ps, wk_t[:, ko, hsl], xn[:, ko, :],
                        start=(ko == 0), stop=(ko == KO - 1),
                    )
                nc.scalar.activation(out=kd[:, h, :], in_=k_ps, func=AF.Copy)
            kt = qk_p.tile([C, TB // C, NH * D], F32, tag="kt")
            vv = qk_p.tile([C, TB // C, NH * D], F32, tag="vv")
            for c in range(TB // C):
                csl = slice(c * C, (c + 1) * C)
                kt_ps = ps_pr.tile([C, NH * D], F32, tag="proj")
                for ko in range(KO):
                    nc.tensor.matmul(
                        kt_ps, xn[:, ko, csl], wk_t[:, ko, :],
                        start=(ko == 0), stop=(ko == KO - 1),
                    )
                nc.vector.tensor_mul(out=kt[:, c, :], in0=kt_ps, in1=ktd_t)
                v_ps = ps_pr.tile([C, NH * D], F32, tag="proj")
                for ko in range(KO):
                    nc.tensor.matmul(
                        v_ps, xn[:, ko, csl], wv_t[:, ko, :],
                        start=(ko == 0), stop=(ko == KO - 1),
                    )
                nc.scalar.activation(out=vv[:, c, :], in_=v_ps, func=AF.Copy)

            # --- retention (chunked recurrence, fp32)
            yt = yt_p.tile([D, NH, TB], F32R, tag="yt")
            for c in range(TB // C):
                csl = slice(c * C, (c + 1) * C)
                at_ps = ps_rt.tile([C, NH * C], F32, tag="ret")
                for h in range(NH):
                    hsl = slice(h * D, (h + 1) * D)
                    nc.tensor.matmul(
                        at_ps[:, h * C:(h + 1) * C],
                        kd[:, h, csl], qs[:, h, csl],
                        start=True, stop=True,
                    )
                amt = sm_p.tile([C, NH * C], F32, tag="amt")
                nc.vector.tensor_mul(out=amt, in0=at_ps, in1=mpr_t)
                y_ps = ps_rt.tile([D, NH * C], F32, tag="ret")
                for h in range(NH):
                    hsl = slice(h * D, (h + 1) * D)
                    nc.tensor.matmul(
                        y_ps[:, h * C:(h + 1) * C],
                        vv[:, c, hsl], amt[:, h * C:(h + 1) * C],
                        start=True, stop=False,
                    )
                    nc.tensor.matmul(
                        y_ps[:, h * C:(h + 1) * C],
                        s_cur[:, hsl], qs[:, h, csl],
                        start=False, stop=True,
                    )
                nc.vector.tensor_copy(
                    out=yt[:, :, csl],
                    in_=y_ps.rearrange("p (h t) -> p h t", h=NH),
                )
                sn_ps = ps_rt.tile([D, NH * D], F32, tag="ret")
                for h in range(NH):
                    hsl = slice(h * D, (h + 1) * D)
                    nc.tensor.matmul(
                        sn_ps[:, hsl], gi_t[:, hsl], s_cur[:, hsl],
                        start=True, stop=False,
                    )
                    nc.tensor.matmul(
                        sn_ps[:, hsl], kt[:, c, hsl], vv[:, c, hsl],
                        start=False, stop=True,
                    )
                nc.vector.tensor_copy(out=s_nxt, in_=sn_ps)
                s_cur, s_nxt = s_nxt, s_cur

            # --- partial output projection pT = yt @ wo_rows (+ flag * x)
            for fo in range(KO):
                fsl = slice(fo * 128, (fo + 1) * 128)
                p_ps = ps_wo.tile([128, TB], F32, tag="wo")
                nc.tensor.matmul(p_ps, fi_t, xt[:, fo, :], start=True, stop=False)
                for eo in range(NH):
                    nc.tensor.matmul(
                        p_ps, wo_t[:, eo, fsl], yt[:, eo, :],
                        start=False, stop=(eo == NH - 1),
                    )
                ptsb = pt_p.tile([128, TB], F32, tag="pt")
                nc.vector.tensor_copy(out=ptsb, in_=p_ps)
                nc.sync.dma_start(out=pt_d.ap()[fsl, tsl], in_=ptsb)

        nc.sync.dma_start(out=so_d.ap(), in_=s_cur)
    nc.compile()
    return nc


# ---------------------------------------------------------------- launch 2


def build_l2():
    SH = T // 2  # tokens per core
    nc = bacc.Bacc("TRN2", target_bir_lowering=False, debug=False)
    p0_d = nc.dram_tensor("p0T", [HD, SH], F32, kind="ExternalInput")
    p1_d = nc.dram_tensor("p1T", [HD, SH], F32, kind="ExternalInput")
    w1_d = nc.dram_tensor("w1p", [HD, P], F32, kind="ExternalInput")
    b1_d = nc.dram_tensor("b1t", [128, P // 128], F32, kind="ExternalInput")
    w2_d = nc.dram_tensor("w2", [P, HD], F32, kind="ExternalInput")
    b2_d = nc.dram_tensor("b2r", [1, HD], F32, kind="ExternalInput")
    out_d = nc.dram_tensor("out", [SH, HD], F32, kind="ExternalOutput")

    HP = 1024                 # tokens per half-pass
    NQ = 4                    # P quarters
    PQ = P // NQ              # 1024
    with tile.TileContext(nc) as tc, ExitStack() as ctx:
        cst = ctx.enter_context(tc.tile_pool(name="cst", bufs=1))
        w1_p = ctx.enter_context(tc.tile_pool(name="w1", bufs=2))
        w2_p = ctx.enter_context(tc.tile_pool(name="w2", bufs=1))
        xn_p = ctx.enter_context(tc.tile_pool(name="xn2", bufs=2))
        st_p = ctx.enter_context(tc.tile_pool(name="stg", bufs=2))
        gh_p = ctx.enter_context(tc.tile_pool(name="ght", bufs=1))
        ob_p = ctx.enter_context(tc.tile_pool(name="osb", bufs=8))
        sm_p = ctx.enter_context(tc.tile_pool(name="sm", bufs=2))
        rw_p = ctx.enter_context(tc.tile_pool(name="rws", bufs=1))
        ic_p = ctx.enter_context(tc.tile_pool(name="ic", bufs=2))
        ps_m1 = ctx.enter_context(tc.tile_pool(name="ps_m1", bufs=2, space="PSUM"))
        ps_o = ctx.enter_context(tc.tile_pool(name="ps_o", bufs=2, space="PSUM"))
        ps_id = ctx.enter_context(tc.tile_pool(name="ps_id", bufs=2, space="PSUM"))
        ps_rw = ctx.enter_context(tc.tile_pool(name="ps_rw", bufs=2, space="PSUM"))

        b1_t = cst.tile([128, P // 128], F32)
        nc.sync.dma_start(out=b1_t, in_=b1_d.ap())
        b2_t = cst.tile([1, HD], F32)
        nc.sync.dma_start(out=b2_t, in_=b2_d.ap())
        id_f = cst.tile([128, 128], F32)
        make_identity(nc, id_f)
        id_r = cst.tile([128, 128], F32R)
        nc.vector.tensor_copy(out=id_r, in_=id_f)
        ones_row = cst.tile([1, 128], F32)
        nc.vector.memset(ones_row, 1.0)
        ones_cr = cst.tile([128, 1], F32R)
        ones_c = cst.tile([128, 1], F32)
        nc.vector.memset(ones_c, 1.0)
        nc.vector.tensor_copy(out=ones_cr, in_=ones_c)
        ones_11 = cst.tile([1, 1], F32)
        nc.vector.memset(ones_11, 1.0)
        eps_t = cst.tile([1, 1], F32)
        nc.vector.memset(eps_t, EPS)

        for hp in range(2):
            # ---------- phase A: x2 = p0 + p1, rstd, normalize (2 blocks)
            xn2 = []
            invc = ic_p.tile([128, 8], F32, tag="invc")
            for blk in range(2):
                t0 = hp * HP + blk * TB
                tsl = slice(t0, t0 + TB)
                xb = xn_p.tile([128, KO, TB], F32R, tag="xn2")
                nc.sync.dma_start(out=xb, in_=_r(_split_ko(p0_d.ap()[:, tsl])))
                for kg in range(2):
                    p1s = st_p.tile([128, 4, TB], F32, tag="stg")
                    nc.sync.dma_start(
                        out=p1s,
                        in_=_split_ko(p1_d.ap()[:, tsl])[:, kg * 4:(kg + 1) * 4, :],
                    )
                    for k4 in range(4):
                        ko = kg * 4 + k4
                        nc.vector.tensor_add(
                            out=xb[:, ko, :], in0=xb[:, ko, :], in1=p1s[:, k4, :]
                        )
                ssq = ps_rw.tile([1, TB], F32, tag="row")
                for ko in range(KO):
                    xsq = sm_p.tile([128, TB], F32R, tag="xsq")
                    nc.vector.tensor_mul(
                        out=xsq, in0=xb[:, ko, :], in1=xb[:, ko, :]
                    )
                    nc.tensor.matmul(
                        ssq, ones_cr, xsq, start=(ko == 0), stop=(ko == KO - 1)
                    )
                istd = rw_p.tile([1, TB], F32, tag="istd")
                nc.scalar.activation(
                    out=istd, in_=ssq, func=AF.Sqrt, bias=eps_t, scale=1.0 / HD
                )
                rstd = rw_p.tile([1, TB], F32, tag="rstd")
                nc.vector.reciprocal(out=rstd, in_=istd)
                rbc = ps_rw.tile([128, TB], F32, tag="row")
                nc.tensor.matmul(rbc, ones_row, rstd, start=True, stop=True)
                for ko in range(KO):
                    nc.vector.tensor_mul(
                        out=xb[:, ko, :], in0=xb[:, ko, :], in1=rbc
                    )
                for tt in range(4):
                    icps = ps_rw.tile([128, 1], F32, tag="row")
                    nc.tensor.matmul(
                        icps, istd[:, tt * 128:(tt + 1) * 128], ones_11,
                        start=True, stop=True,
                    )
                    nc.vector.tensor_copy(
                        out=invc[:, blk * 4 + tt:blk * 4 + tt + 1], in_=icps
                    )
                xn2.append(xb)

            # osb accumulators for this half-pass (8 token-tiles x full HD)
            osb = [
                ob_p.tile([128, HD], F32, tag="osb", name=f"osb_{hp}_{i}")
                for i in range(8)
            ]

            # ---------- phase B: quarters of P
            for q in range(4):
                qsl = slice(q * PQ, (q + 1) * PQ)
                w1t = w1_p.tile([128, KO, PQ], F32R, tag="w1")
                nc.sync.dma_start(out=w1t, in_=_r(_split_ko(w1_d.ap()[:, qsl])))
                w2t = w2_p.tile([128, PQ // 128, HD], F32R, tag="w2")
                nc.sync.dma_start(
                    out=w2t,
                    in_=_r(w2_d.ap()[qsl, :].rearrange("(po kp) f -> kp po f", kp=128)),
                )
                for blk in range(2):
                    ght = gh_p.tile([128, PQ // 128, TB], F32R, tag="ght")
                    for po in range(PQ // 128):
                        h_ps = ps_m1.tile([128, TB], F32, tag="m1")
                        for ko in range(KO):
                            nc.tensor.matmul(
                                h_ps, w1t[:, ko, po * 128:(po + 1) * 128],
                                xn2[blk][:, ko, :],
                                start=(ko == 0), stop=(ko == KO - 1),
                            )
                        nc.scalar.activation(
                            out=ght[:, po, :], in_=h_ps, func=AF.Gelu_apprx_tanh,
                            bias=b1_t[:, q * 8 + po:q * 8 + po + 1], scale=1.0,
                        )
                    for tt in range(4):
                        ti = blk * 4 + tt
                        ttsl = slice(tt * 128, (tt + 1) * 128)
                        for fs in range(2):
                            fssl = slice(fs * 512, (fs + 1) * 512)
                            o_ps = ps_o.tile([128, 512], F32, tag="o")
                            if q == 0:
                                nc.tensor.matmul(
                                    o_ps, ones_row, b2_t[:, fssl],
                                    start=True, stop=False,
                                )
                            for po in range(PQ // 128):
                                nc.tensor.matmul(
                                    o_ps, ght[:, po, ttsl], w2t[:, po, fssl],
                                    start=(q != 0 and po == 0),
                                    stop=(po == PQ // 128 - 1),
                                )
                            if q == 0:
                                id_ps = ps_id.tile([128, 512], F32, tag="id")
                                for jf in range(4):
                                    nc.tensor.matmul(
                                        id_ps[:, jf * 128:(jf + 1) * 128],
                                        xn2[blk][:, fs * 4 + jf, ttsl], id_r,
                                        start=True, stop=True,
                                    )
                                nc.scalar.activation(
                                    out=osb[ti][:, fssl], in_=id_ps, func=AF.Copy,
                                    scale=invc[:, ti:ti + 1],
                                )
                            nc.vector.tensor_add(
                                out=osb[ti][:, fssl], in0=osb[ti][:, fssl],
                                in1=o_ps,
                            )
                            if q == 3:
                                t0 = hp * HP + blk * TB + tt * 128
                                nc.sync.dma_start(
                                    out=out_d.ap()[t0:t0 + 128, fssl],
                                    in_=osb[ti][:, fssl],
                                )
    nc.compile()
    return nc


# ---------------------------------------------------------------- host glue


def _gammas():
    return 1.0 - np.exp2(-5.0 - np.arange(H, dtype=np.float64))


def _l1_inputs(x, state, norm1_w, wq, wk, wv, wo):
    gam = _gammas()
    sarange = np.arange(C, dtype=np.float64)
    in_maps = []
    n1 = norm1_w.astype(np.float64)[:, None]
    for b in range(B):
        xT = np.ascontiguousarray(x[b].T).astype(np.float32)
        for g in range(2):
            cols = slice(g * NH * D, (g + 1) * NH * D)
            gh = gam[g * NH:(g + 1) * NH]
            wqp = (n1 * wq[:, cols].astype(np.float64) * D ** -0.5).astype(np.float32)
            wkp = (n1 * wk[:, cols].astype(np.float64)).astype(np.float32)
            wvp = (n1 * wv[:, cols].astype(np.float64)).astype(np.float32)
            wop = np.ascontiguousarray(wo[cols, :]).astype(np.float32)
            s0 = np.ascontiguousarray(
                state[b, g * NH:(g + 1) * NH].transpose(1, 0, 2).reshape(D, NH * D)
            ).astype(np.float32)
            # qsm[p, h, tl] = gam_h^((tl % C) + 1)
            tl = np.arange(TB, dtype=np.float64) % C + 1.0
            qsm = np.broadcast_to(
                (gh[:, None] ** tl[None, :])[None, :, :], (128, NH, TB)
            ).astype(np.float32)
            # mpr[s, h*C + t] = gam_h^(-(s+1)) if s <= t else 0
            svec = sarange[:, None]
            tvec = sarange[None, :]
            mpr = np.zeros((C, NH * C), np.float64)
            for hh in range(NH):
                blkm = np.where(svec <= tvec, gh[hh] ** (-(svec + 1.0)), 0.0)
                mpr[:, hh * C:(hh + 1) * C] = blkm
            mpr = mpr.astype(np.float32)
            # ktd[s, h*D + d] = gam_h^(C-1-s)
            ktd = np.repeat(
                gh[None, :] ** (C - 1.0 - sarange)[:, None], D, axis=1
            ).reshape(C, NH * D).astype(np.float32)
            # gI[d, h*D + e] = gam_h^C * I
            gI = np.zeros((D, NH * D), np.float32)
            for hh in range(NH):
                gI[:, hh * D:(hh + 1) * D] = np.eye(D) * (gh[hh] ** C)
            fI = (np.eye(128) * (1.0 if g == 0 else 0.0)).astype(np.float32)
            in_maps.append({
                "xT": xT, "wqp": wqp, "wkp": wkp, "wvp": wvp, "wop": wop,
                "s0": s0, "qsm": np.ascontiguousarray(qsm), "mpr": mpr,
                "ktd": ktd, "gI": gI, "fI": fI,
            })
    return in_maps


def _l2_inputs(p_parts, norm2_w, w1, b1, w2, b2):
    w1p = (norm2_w.astype(np.float64)[:, None] * w1.astype(np.float64)).astype(
        np.float32
    )
    b1t = np.ascontiguousarray(b1.reshape(P // 128, 128).T).astype(np.float32)
    b2r = b2.reshape(1, HD).astype(np.float32)
    w2c = np.ascontiguousarray(w2).astype(np.float32)
    in_maps = []
    SH = T // 2
    for b in range(B):
        for half in range(2):
            tsl = slice(half * SH, (half + 1) * SH)
            in_maps.append({
                "p0T": np.ascontiguousarray(p_parts[b][0][:, tsl]),
                "p1T": np.ascontiguousarray(p_parts[b][1][:, tsl]),
                "w1p": w1p, "b1t": b1t, "w2": w2c, "b2r": b2r,
            })
    return in_maps


def kernel(x, state, norm1_w, wq, wk, wv, wo, norm2_w, w1, b1, w2, b2):
    if "l1" not in _cache:
        _cache["l1"] = build_l1()
    if "l2" not in _cache:
        _cache["l2"] = build_l2()
    nc1, nc2 = _cache["l1"], _cache["l2"]

    x = np.asarray(x, np.float32)
    state = np.asarray(state, np.float32)

    in1 = _l1_inputs(x, state, norm1_w, wq, wk, wv, wo)
    r1 = bass_utils.run_bass_kernel_spmd(nc1, in1, core_ids=list(range(8)))

    new_state = np.empty((B, H, D, D), np.float32)
    p_parts = []
    for b in range(B):
        parts = []
        for g in range(2):
            res = r1.results[b * 2 + g]
            parts.append(res["pT"])
            new_state[b, g * NH:(g + 1) * NH] = (
                res["sout"].reshape(D, NH, D).transpose(1, 0, 2)
            )
        p_parts.append(parts)

    in2 = _l2_inputs(p_parts, norm2_w, w1, b1, w2, b2)
    r2 = bass_utils.run_bass_kernel_spmd(nc2, in2, core_ids=list(range(8)))

    out = np.empty((B, T, HD), np.float32)
    SH = T // 2
    for b in range(B):
        for half in range(2):
            out[b, half * SH:(half + 1) * SH] = r2.results[b * 2 + half]["out"]
    return out, new_state
